# revision 11
# baseline (speedup 1.0000x reference)
"""ARMA GNN (2x ARMAConv K=2 T=2 + mean-pool + FC head) on 8 TRN2 NeuronCores.

Strategy (graph/data parallel, transposed aggregation), v2.2:
- Factorize the GCN norm: norm[e] = dinv[src]*dinv[dst]; edge aggregation is a
  binary scatter-add of src-prescaled rows, re-scaled on the dst side.
- Host: permute nodes into 784 degree-balanced blocks of 128 (98/core); each
  edge is assigned to its dst 64-block. Per (64-block, src-quartile) lane
  quota = max over cores keeps the SPMD instruction stream identical on all 8
  cores with ~3% padding. Within each gather group (<=8 blocks, split at
  quartile boundaries so quota balancing survives) the node order is w-major
  (w*nblk+bi), so selector one-hots build with fully packed APs (2x DVE mode,
  one is_equal per group per round) and the scatter matmuls write strided
  3D PSUM views whose column order equals node order.
- Per round (4 = 2 convs x T=2): node-major Z matmul per 128-tile (stationary
  = feature-major state slice, moving = weights) -> PSUM->SBUF copy on Act
  with fused per-node dinv scale -> quartile-run-batched ag_in writes -> bf16
  AllGather of Z' [100352,128] -> per (group x quartile) one dma_gather
  (int16 over a <=25600-row table view) -> one matmul per 128-edge chunk:
  stationary = gathered rows, moving = w-major selector slice -> post in
  transposed space: relu(dinv*P^T + root^T + bias^T); Lrelu activation folds
  the leaky-relu between convs.
- Epilogue: segment-sum pooling via one-hot matmuls, AllReduce [65,64],
  FC head + sigmoid on device. Output [64] f32.
- Cost-model status (TimelineSim, collectives stubbed): ~1.17ms sim; DMA
  engines ~94% busy, dominated by the per-edge dma_gather descriptor floor
  (256B granule, ~1.42ns/edge/round); PE sequencer dispatch (~2 instrs per
  128-edge chunk) is co-critical.
"""

import numpy as np
import ml_dtypes

NCORES = 8
N = 100000
F = 128
H = 64
G = 64
S = 12544                 # nodes per core = 98*128
PN = NCORES * S           # 100352 padded global nodes
NB = 98                   # 128-node tiles per core (Z phase / pooling)
QR = PN // 4              # 25088 rows per quartile table view
BW = 64                   # aggregation block width (rows per one-hot)
GSIZE = 8                 # aggregation blocks per gather group (512 nodes)
PADROW = 999.0            # dstrow sentinel for masked lanes
STUB_COLLECTIVES = False  # replace collectives with local DMA (cost-model runs)


def _groups():
    # groups never cross quartile boundaries: the w-major permutation must
    # keep every node in its quartile so pass-2 quota balancing stays valid.
    nbw = S // BW
    qsz0 = ((S + 4 * 128 - 1) // (4 * 128)) * 128
    qb = sorted({min(qsz0 * k // BW, nbw) for k in range(1, 4)} | {nbw})
    gs, b = [], 0
    for e in qb:
        while b < e:
            gs.append(list(range(b, min(b + GSIZE, e))))
            b = min(b + GSIZE, e)
    return gs


def _wmajor_pi():
    """Local-position permutation: within each group of GSIZE 64-blocks the
    node order becomes w-major (w*nblk + bi), so PSUM columns written by the
    w-major selector matmuls coincide with node order."""
    pi = np.empty(S, np.int64)
    groups = _groups()
    for g, blks in enumerate(groups):
        nblk = len(blks)
        base = blks[0] * BW
        idx = np.arange(nblk * BW)
        bi = idx // BW
        w = idx % BW
        pi[base + idx] = base + w * nblk + bi
    return pi


def _preprocess(x, edge_index, batch):
    src, dst = edge_index[0].astype(np.int64), edge_index[1].astype(np.int64)
    deg = np.bincount(dst, minlength=N).astype(np.int64)

    # node permutation: serpentine deal by degree over NCORES*NB bins
    order = np.argsort(-deg, kind="stable")
    order = np.concatenate([order, np.arange(N, PN)])
    nbins = NCORES * NB
    new_of_old = np.empty(PN, np.int64)
    idx = np.arange(PN)
    r = idx // nbins
    pos = idx % nbins
    binid = np.where(r % 2 == 0, pos, nbins - 1 - pos)
    new_of_old[order] = (binid // NB) * S + (binid % NB) * 128 + r

    old_of_new = np.empty(PN, np.int64)
    old_of_new[new_of_old] = np.arange(PN)

    # ---- pass 2: re-deal nodes within their (core, quarter) range to
    # balance per-(64-block, src-quarter) edge-count 4-vectors; quarter
    # membership (and thus every edge's table choice) is fixed by pass 1,
    # so this is non-circular. Shrinks the max-over-cores quotas. ----
    qsz0_p = ((S + 4 * 128 - 1) // (4 * 128)) * 128
    qszs_p = []
    rem_p = S
    for _ in range(4):
        t = min(qsz0_p, rem_p)
        qszs_p.append(t)
        rem_p -= t
    qoffs_p = [0, qszs_p[0], qszs_p[0] + qszs_p[1],
               qszs_p[0] + qszs_p[1] + qszs_p[2]]
    nq_p = len([t for t in qszs_p if t > 0])
    q_e = np.minimum((new_of_old[src] % S) // qsz0_p, nq_p - 1)
    degq = np.zeros((PN, 4), np.int64)
    np.add.at(degq, (dst, q_e), 1)

    new2 = np.empty(PN, np.int64)
    for c in range(NCORES):
        for j in range(nq_p):
            if qszs_p[j] == 0:
                continue
            base = c * S + qoffs_p[j]
            olds = old_of_new[base:base + qszs_p[j]]
            v = degq[olds].astype(np.float64)           # [m, 4]
            nblk = qszs_p[j] // BW
            sums = np.zeros((nblk, 4))
            slots = np.full(nblk, BW, np.int64)
            fill = np.zeros(nblk, np.int64)
            order2 = np.argsort(-v.sum(axis=1), kind="stable")
            for oi in order2:
                cand = sums + v[oi]                      # [nblk, 4]
                score = cand.max(axis=1)
                score[slots == 0] = np.inf
                b = int(np.argmin(score))
                new2[olds[oi]] = base + b * BW + fill[b]
                sums[b] += v[oi]
                slots[b] -= 1
                fill[b] += 1

    # compose the w-major within-group permutation
    pi = _wmajor_pi()
    new_of_old = (new2 // S) * S + pi[new2 % S]
    old_of_new = np.empty(PN, np.int64)
    old_of_new[new_of_old] = np.arange(PN)

    x_pad = np.zeros((PN, F), np.float32)
    x_pad[:N] = x
    x_new = x_pad[old_of_new]
    deg_pad = np.zeros(PN, np.float32)
    deg_pad[:N] = deg.astype(np.float32)
    deg_new = deg_pad[old_of_new]
    batch_pad = np.full(PN, PADROW, np.float32)
    batch_pad[:N] = batch.astype(np.float32)
    batch_new = batch_pad[old_of_new]

    per_core_xT = [np.ascontiguousarray(x_new[c * S:(c + 1) * S].T)
                   .astype(ml_dtypes.bfloat16) for c in range(NCORES)]
    dinv_new = np.where(deg_new > 0,
                        1.0 / np.sqrt(np.maximum(deg_new, 1.0)),
                        0.0).astype(np.float32)
    # dinv broadcast [128, S] bf16 (same value down each column)
    per_core_dinvbc = [
        np.ascontiguousarray(
            np.broadcast_to(dinv_new[c * S:(c + 1) * S][None, :], (128, S)))
        .astype(ml_dtypes.bfloat16) for c in range(NCORES)]
    # dinv node-major per tile: [128, NB] f32, dinv_col[p, t] = dinv[t*128+p]
    per_core_dinvcol = [np.ascontiguousarray(
        dinv_new[c * S:(c + 1) * S].reshape(NB, 128).T) for c in range(NCORES)]
    per_core_batch = [np.ascontiguousarray(batch_new[c * S:(c + 1) * S]
                                           .reshape(NB, 128).T)
                      for c in range(NCORES)]                      # [128, NB]

    # edges
    nsrc = new_of_old[src]
    ndst = new_of_old[dst]
    owner = ndst // S
    groups = _groups()
    ngrp = len(groups)
    nblk_of_g = np.array([len(b) for b in groups], np.int64)
    gbase = np.array([b[0] * BW for b in groups], np.int64)
    loc = ndst % S
    gid = np.searchsorted(gbase, loc, side="right") - 1
    off = loc - gbase[gid]
    nblk_e = nblk_of_g[gid]
    bi = off % nblk_e
    row = (off // nblk_e).astype(np.float32)            # w within 64-block
    blk = gbase[gid] // BW + bi                         # global 64-block id

    qsz0 = ((S + 4 * 128 - 1) // (4 * 128)) * 128
    qszs = []
    rem = S
    for _ in range(4):
        t = min(qsz0, rem)
        qszs.append(t)
        rem -= t
    qoffs = [0, qszs[0], qszs[0] + qszs[1], qszs[0] + qszs[1] + qszs[2]]
    nq = len([t for t in qszs if t > 0])
    local = nsrc % S
    rank = nsrc // S
    q = np.minimum(local // qsz0, nq - 1)
    qsz_a = np.array(qszs, np.int64)
    qoff_a = np.array(qoffs, np.int64)
    qidx = (rank * qsz_a[q] + (local - qoff_a[q])).astype(np.int16)

    nbw = S // BW
    key = (owner * nbw + blk) * 4 + q
    counts = np.bincount(key, minlength=NCORES * nbw * 4).reshape(NCORES, nbw, 4)
    quota = counts.max(axis=0)                                     # [nbw, 4]

    layout = []     # per (g,q): blocks, per-block lane offsets, n (=sum quota)
    for g, blks in enumerate(groups):
        for qq in range(4):
            offs, sacc = [], 0
            for b in blks:
                offs.append(sacc)
                sacc += int(quota[b, qq])
            nidx = ((sacc + 127) // 128) * 128
            layout.append(dict(g=g, q=qq, blocks=blks, offs=offs,
                               n=sacc, nidx=nidx))

    # covers ordered by (chunk, block); per chunk the covers are contiguous
    # blocks -> one matmul per chunk spanning its covers.
    covers = []      # per (g,q): list of (c, b, lo, hi, col)
    chunk_mms = []   # per (g,q): list of (c, col0, ncov, bi0)
    ncols_total = 0
    for lay in layout:
        lst = []
        for bi_, b in enumerate(lay["blocks"]):
            lo = lay["offs"][bi_]
            hi = lo + int(quota[b, lay["q"]])
            if hi == lo:
                continue
            c0, c1 = lo // 128, (hi - 1) // 128
            for c in range(c0, c1 + 1):
                lst.append((c, b, lo, hi))
        lst.sort(key=lambda t: (t[0], t[1]))
        lst2 = []
        for i, (c, b, lo, hi) in enumerate(lst):
            lst2.append((c, b, lo, hi, ncols_total + i))
        ncols_total += len(lst2)
        covers.append(lst2)
        mms = []
        i = 0
        while i < len(lst2):
            j = i
            while (j + 1 < len(lst2) and lst2[j + 1][0] == lst2[i][0]):
                j += 1
            c = lst2[i][0]
            bi0 = lay["blocks"].index(lst2[i][1])
            # covered blocks must be contiguous in bi for one matmul
            mms.append((c, lst2[i][4], j - i + 1, bi0))
            i = j + 1
        chunk_mms.append(mms)

    # per-group contiguous cover/col spans for the merged sel build
    grp_cols = []   # per g: (c0, ncv)
    for g in range(ngrp):
        lis = [g * 4 + qq for qq in range(4)]
        cs = [c[0][4] for li in lis if (c := covers[li])]
        ncv = sum(len(covers[li]) for li in lis)
        grp_cols.append((min(cs) if cs else 0, ncv))

    # per-core edge fill
    eorder = np.lexsort((q, blk, owner))
    so_q = q[eorder]
    so_qidx = qidx[eorder]
    so_row = row[eorder]
    keysort = (owner[eorder] * nbw + blk[eorder]) * 4 + so_q
    kstart = np.searchsorted(keysort, np.arange(NCORES * nbw * 4))

    tot_nidx = sum(l["nidx"] for l in layout)
    per_core_idx, per_core_dst = [], []
    for c in range(NCORES):
        idx_parts = []
        dstcols = np.full((128, ncols_total), PADROW, np.float32)
        for li, lay in enumerate(layout):
            flat = np.zeros(lay["nidx"], np.int16)
            lane_dst = np.full(lay["nidx"], PADROW, np.float32)
            for bi_, b in enumerate(lay["blocks"]):
                kk = (c * nbw + b) * 4 + lay["q"]
                s0 = kstart[kk]
                cnt = int(counts[c, b, lay["q"]])
                lo = lay["offs"][bi_]
                flat[lo:lo + cnt] = so_qidx[s0:s0 + cnt]
                lane_dst[lo:lo + cnt] = so_row[s0:s0 + cnt]
            idx_parts.append(np.tile(flat.reshape(-1, 16).T, (8, 1)))
            for (cc, b, lo, hi, col) in covers[li]:
                colv = np.full(128, PADROW, np.float32)
                sl = max(lo, cc * 128)
                e = min(hi, (cc + 1) * 128)
                colv[sl - cc * 128:e - cc * 128] = lane_dst[sl:e]
                dstcols[:, col] = colv
        per_core_idx.append(np.concatenate(idx_parts, axis=1))
        per_core_dst.append(dstcols.astype(ml_dtypes.bfloat16))

    maxcv_g = max(ncv for _, ncv in grp_cols)
    # iota3 [128, 64, maxcv_g] bf16: value w, constant along cv
    iota3 = np.ascontiguousarray(
        np.broadcast_to(np.arange(BW, dtype=np.float32)[None, :, None],
                        (128, BW, maxcv_g))).astype(ml_dtypes.bfloat16)

    sched = dict(layout=layout, covers=covers, chunk_mms=chunk_mms,
                 ncols=ncols_total, tot_nidx=tot_nidx, groups=groups,
                 qszs=qszs, qoffs=qoffs, grp_cols=grp_cols, maxcv_g=maxcv_g)
    data = dict(xT=per_core_xT, dinvbc=per_core_dinvbc,
                dinvcol=per_core_dinvcol, batch=per_core_batch,
                idx=per_core_idx, dst=per_core_dst, iota3=iota3)
    return sched, data


def _pack_weights(iw1, w1, rw1, b1, iw2, w2, rw2, b2, fcw, fcb):
    # wbig [128, 3*128] f32: Wz1 | Wr1 | Wr2   (lhsT, contract = F)
    wz1 = np.concatenate([iw1[0], iw1[1]], axis=1)
    wr1 = np.concatenate([rw1[0, 0], rw1[0, 1]], axis=1)
    wr2 = np.concatenate([rw1[1, 0], rw1[1, 1]], axis=1)
    wbig = np.concatenate([wz1, wr1, wr2], axis=1).astype(np.float32)

    def bd(w):
        m = np.zeros((128, 128), np.float32)
        m[0:64, 0:64] = w[0, 0]
        m[64:128, 64:128] = w[0, 1]
        return m

    wbd = np.concatenate([bd(w1), bd(w2)], axis=1).astype(np.float32)  # [128,256]
    wz3 = np.concatenate([iw2[0], iw2[1]], axis=1)
    wr3 = np.concatenate([rw2[0, 0], rw2[0, 1]], axis=1)
    wr4 = np.concatenate([rw2[1, 0], rw2[1, 1]], axis=1)
    wsml = np.concatenate([wz3, wr3, wr4], axis=1).astype(np.float32)  # [64,384]
    biasT = np.stack([b1[0].ravel(), b1[1].ravel(),
                      b2[0].ravel(), b2[1].ravel()], axis=1)           # [128,4]
    khalf = np.zeros((128, 64), np.float32)
    khalf[0:64] = 0.5 * np.eye(64)
    khalf[64:128] = 0.5 * np.eye(64)
    return (wbig, wsml, wbd, biasT.astype(np.float32), khalf,
            fcw.astype(np.float32), fcb.reshape(1, 1).astype(np.float32))


def _build(sched):
    import concourse.bass as bass
    import concourse.bacc as bacc
    import concourse.mybir as mybir
    import concourse.tile as tile
    from concourse.masks import make_identity

    f32 = mybir.dt.float32
    bf16 = mybir.dt.bfloat16
    i16 = mybir.dt.int16
    Alu = mybir.AluOpType
    Act = mybir.ActivationFunctionType

    layout = sched["layout"]
    covers = sched["covers"]
    chunk_mms = sched["chunk_mms"]
    ncols = sched["ncols"]
    groups = sched["groups"]
    grp_cols = sched["grp_cols"]
    maxcv_g = sched["maxcv_g"]
    maxch = max(l["nidx"] // 128 for l in layout)

    nc = bacc.Bacc("TRN2", target_bir_lowering=False, debug=False,
                   num_devices=1 if STUB_COLLECTIVES else NCORES)

    xT_d = nc.dram_tensor("xT", [128, S], bf16, kind="ExternalInput")
    dinvbc_d = nc.dram_tensor("dinvbc", [128, S], bf16, kind="ExternalInput")
    dinvcol_d = nc.dram_tensor("dinvcol", [128, NB], f32, kind="ExternalInput")
    bat_d = nc.dram_tensor("batch", [128, NB], f32, kind="ExternalInput")
    idx_d = nc.dram_tensor("idx", [128, sched["tot_nidx"] // 16], i16,
                           kind="ExternalInput")
    dstr_d = nc.dram_tensor("dstr", [128, ncols], bf16, kind="ExternalInput")
    iota3_d = nc.dram_tensor("iota3", [128, BW * maxcv_g], bf16,
                             kind="ExternalInput")
    wbig_d = nc.dram_tensor("wbig", [128, 384], f32, kind="ExternalInput")
    wsml_d = nc.dram_tensor("wsml", [64, 384], f32, kind="ExternalInput")
    wbd_d = nc.dram_tensor("wbd", [128, 256], f32, kind="ExternalInput")
    biasT_d = nc.dram_tensor("biasT", [128, 4], f32, kind="ExternalInput")
    khalf_d = nc.dram_tensor("khalf", [128, 64], f32, kind="ExternalInput")
    fcw_d = nc.dram_tensor("fcw", [64, 1], f32, kind="ExternalInput")
    fcb_d = nc.dram_tensor("fcb", [1, 1], f32, kind="ExternalInput")
    out_d = nc.dram_tensor("out", [1, G], f32, kind="ExternalOutput")

    qszs = sched["qszs"]
    qoffs = sched["qoffs"]
    ag_in = [[nc.dram_tensor(f"ag_in{r}_{j}", [qszs[j], 128], bf16)
              if qszs[j] > 0 else None for j in range(4)] for r in range(4)]
    ag_out = [[nc.dram_tensor(f"ag_out{r}_{j}", [NCORES * qszs[j], 128], bf16,
                              addr_space="Shared")
               if qszs[j] > 0 else None for j in range(4)] for r in range(4)]
    ar_in = nc.dram_tensor("ar_in", [65, G], f32)
    ar_out = nc.dram_tensor("ar_out", [65, G], f32, addr_space="Shared")

    with tile.TileContext(nc) as tc:
        with (
            tc.tile_pool(name="big", bufs=1) as big,
            tc.tile_pool(name="land", bufs=2) as land,
            tc.tile_pool(name="selpool", bufs=2) as selp,
            tc.tile_pool(name="work", bufs=4) as work,
            tc.tile_pool(name="zsb", bufs=3) as zsbp,
            tc.tile_pool(name="psG", bufs=2, space="PSUM") as psG,
            tc.tile_pool(name="psR", bufs=2, space="PSUM") as psR,
            tc.tile_pool(name="psZ", bufs=2, space="PSUM") as psZ,
            tc.tile_pool(name="psP", bufs=1, space="PSUM") as psP,
        ):
            # ---------- prologue ----------
            xT = big.tile([128, S], bf16, tag="xT")
            for pc in range(4):
                sl = slice(pc * (S // 4), (pc + 1) * (S // 4))
                nc.sync.dma_start(out=xT[:, sl], in_=xT_d[:, sl])
            outT = big.tile([128, S], bf16, tag="outT")
            h1T = big.tile([64, S], bf16, tag="h1T")
            dstr = big.tile([128, ncols], bf16, tag="dstr")
            nc.sync.dma_start(out=dstr[:], in_=dstr_d[:])
            iota3 = big.tile([128, BW, maxcv_g], bf16, tag="iota3")
            nc.sync.dma_start(
                out=bass.AP(iota3[:].tensor, iota3[:].offset,
                            [[iota3[:].ap[0][0], 128], [1, BW * maxcv_g]]),
                in_=iota3_d[:])
            bat = big.tile([128, NB], f32, tag="bat")
            nc.sync.dma_start(out=bat[:], in_=bat_d[:])
            dinv_col = big.tile([128, NB], f32, tag="dinvcol")
            nc.sync.dma_start(out=dinv_col[:], in_=dinvcol_d[:])
            wbig = big.tile([128, 384], f32, tag="wbig")
            nc.sync.dma_start(out=wbig[:], in_=wbig_d[:])
            wsml = big.tile([64, 384], f32, tag="wsml")
            nc.sync.dma_start(out=wsml[:], in_=wsml_d[:])
            wbd = big.tile([128, 256], f32, tag="wbd")
            nc.sync.dma_start(out=wbd[:], in_=wbd_d[:])
            biasT = big.tile([128, 4], f32, tag="biasT")
            nc.sync.dma_start(out=biasT[:], in_=biasT_d[:])
            khalf = big.tile([128, 64], f32, tag="khalf")
            nc.sync.dma_start(out=khalf[:], in_=khalf_d[:])
            fcw = big.tile([64, 1], f32, tag="fcw")
            nc.sync.dma_start(out=fcw[:], in_=fcw_d[:])
            fcb = big.tile([1, 1], f32, tag="fcb")
            nc.sync.dma_start(out=fcb[:], in_=fcb_d[:])
            wbig_b = big.tile([128, 384], bf16, tag="wbig_b")
            nc.vector.tensor_copy(wbig_b[:], wbig[:])
            wsml_b = big.tile([64, 384], bf16, tag="wsml_b")
            nc.vector.tensor_copy(wsml_b[:], wsml[:])
            wbd_b = big.tile([128, 256], bf16, tag="wbd_b")
            nc.vector.tensor_copy(wbd_b[:], wbd[:])

            iota_i = big.tile([128, 128], mybir.dt.int32, tag="iota_i")
            nc.gpsimd.iota(iota_i[:], pattern=[[1, 128]], base=0,
                           channel_multiplier=0)
            iota_g = big.tile([128, G], f32, tag="iota_g")
            nc.vector.tensor_copy(iota_g[:], iota_i[:, :G])
            identf = big.tile([64, 64], f32, tag="identf")
            make_identity(nc, identf[:])

            dinv_bc = big.tile([128, S], bf16, tag="dinv_bc")
            nc.sync.dma_start(out=dinv_bc[:], in_=dinvbc_d[:])

            pool_ps = psP.tile([65, G], f32, space="PSUM", tag="poolacc")

            # ---------- rounds ----------
            for r in range(4):
                # Z phase: per 128-node tile, node-major Z matmul; PSUM->SBUF
                # on Act with fused dinv scale; quad-batched ag_in writes.
                for b0 in range(0, NB, 4):
                    nt4 = min(4, NB - b0)
                    zq = zsbp.tile([128, 4, 128], bf16, tag="zq")
                    for j in range(nt4):
                        t = b0 + j
                        tc_sl = slice(t * 128, (t + 1) * 128)
                        zp = psZ.tile([128, 128], f32, space="PSUM", tag="z")
                        if r == 0:
                            nc.tensor.matmul(out=zp[:], lhsT=xT[:, tc_sl],
                                             rhs=wbig_b[:, 0:128],
                                             start=True, stop=True)
                        elif r == 2:
                            nc.tensor.matmul(out=zp[:], lhsT=h1T[:, tc_sl],
                                             rhs=wsml_b[:, 0:128],
                                             start=True, stop=True)
                        else:
                            wof = 0 if r == 1 else 128
                            nc.tensor.matmul(out=zp[:], lhsT=outT[:, tc_sl],
                                             rhs=wbd_b[:, wof:wof + 128],
                                             start=True, stop=True)
                        nc.scalar.activation(zq[:, j, :], zp[:], Act.Copy,
                                             scale=dinv_col[:, t:t + 1])
                    # one DMA per quartile-run: DRAM rows ro+j*128+p <- zq[p,j,:]
                    j0 = 0
                    while j0 < nt4:
                        jq = min(((b0 + j0) * 128) // max(qszs[0], 1), 3)
                        j1 = j0
                        while (j1 + 1 < nt4 and
                               min(((b0 + j1 + 1) * 128) // max(qszs[0], 1), 3)
                               == jq):
                            j1 += 1
                        ro = (b0 + j0) * 128 - qoffs[jq]
                        nrun = j1 - j0 + 1
                        tgt = ag_in[r][jq]
                        out_ap = bass.AP(
                            tgt[:].tensor, tgt[:].offset + ro * 128,
                            [[128, 128], [128 * 128, nrun], [1, 128]])
                        nc.sync.dma_start(out=out_ap, in_=zq[:, j0:j1 + 1, :])
                        j0 = j1 + 1

                for j in range(4):
                    if ag_in[r][j] is None:
                        continue
                    if STUB_COLLECTIVES:
                        # pair-row views (512B elems) avoid the <512B DMA
                        # penalty; same bytes as the row-major copy.
                        src = ag_in[r][j][:]
                        dst = ag_out[r][j][:]
                        nc.sync.dma_start(
                            out=bass.AP(dst.tensor, dst.offset,
                                        [[256, qszs[j] // 2], [1, 256]]),
                            in_=bass.AP(src.tensor, src.offset,
                                        [[256, qszs[j] // 2], [1, 256]]))
                    else:
                        nc.gpsimd.collective_compute(
                            "AllGather", mybir.AluOpType.bypass,
                            replica_groups=[list(range(NCORES))],
                            ins=[ag_in[r][j][:]], outs=[ag_out[r][j][:]])

                ioff16 = 0
                for g, blks in enumerate(groups):
                    nb_g = len(blks)
                    pG = psG.tile([128, GSIZE * BW], f32, space="PSUM",
                                  tag="Gp")
                    nc.vector.memset(pG[:, 0:nb_g * BW], 0.0)
                    gn16 = sum(layout[g * 4 + qq]["nidx"] // 16
                               for qq in range(4))
                    it_g = work.tile([128, 4 * (maxch * 128) // 16], i16,
                                     tag="idxg")
                    if gn16 > 0:
                        nc.sync.dma_start(out=it_g[:, 0:gn16],
                                          in_=idx_d[:, ioff16:ioff16 + gn16])

                    # merged w-major selector build for the whole group:
                    # sel[p, w, cv] = (w == dstr[p, c0+cv]); all APs packed.
                    c0g, ncvg = grp_cols[g]
                    sel = selp.tile([128, BW, maxcv_g], bf16, tag="sel")
                    if ncvg:
                        dbc = dstr[:, c0g:c0g + ncvg]
                        dbc3 = bass.AP(dbc.tensor, dbc.offset,
                                       [[dbc.ap[0][0], 128], [0, BW],
                                        [1, ncvg]])
                        nc.vector.tensor_tensor(out=sel[:, :, 0:ncvg],
                                                in0=iota3[:, :, 0:ncvg],
                                                in1=dbc3, op=Alu.is_equal)

                    goff16 = 0
                    gts = []
                    for qq in range(4):
                        li = g * 4 + qq
                        lay = layout[li]
                        nch = lay["nidx"] // 128
                        n16 = lay["nidx"] // 16
                        gt = land.tile([128, maxch, 128], bf16, tag=f"g{qq}")
                        if nch > 0:
                            nc.gpsimd.dma_gather(
                                out_ap=gt[:, 0:nch, :],
                                in_ap=ag_out[r][lay["q"]][:],
                                idxs_ap=it_g[:, goff16:goff16 + n16],
                                num_idxs=lay["nidx"],
                                num_idxs_reg=lay["nidx"],
                                elem_size=128,
                                single_packet=False,
                            )
                        goff16 += n16
                        gts.append(gt)
                        ioff16 += n16

                    pGv = pG[:]
                    for qq in range(4):
                        li = g * 4 + qq
                        if not covers[li]:
                            continue
                        for (c, col, ncov, bi0) in chunk_mms[li]:
                            out_ap = bass.AP(
                                pGv.tensor, pGv.offset + bi0,
                                [[pGv.ap[0][0], 128], [nb_g, BW], [1, ncov]])
                            nc.tensor.matmul(
                                out=out_ap,
                                lhsT=gts[qq][:, c, :],
                                rhs=sel[:, :, col - c0g:col - c0g + ncov],
                                start=False, stop=False,
                                skip_group_check=True)

                    b = blks[0]
                    w = nb_g * BW
                    bc = slice(b * BW, b * BW + w)
                    pR = psR.tile([128, 512], f32, space="PSUM", tag="R")
                    if r <= 1:
                        nc.tensor.matmul(
                            out=pR[:, 0:w],
                            lhsT=wbig_b[:, 128 + r * 128:256 + r * 128],
                            rhs=xT[:, bc], start=True, stop=True)
                    else:
                        wof = 128 + (r - 2) * 128
                        nc.tensor.matmul(
                            out=pR[:, 0:w], lhsT=wsml_b[:, wof:wof + 128],
                            rhs=h1T[:, bc], start=True, stop=True)
                    t1 = work.tile([128, 512], f32, tag="t1")
                    nc.vector.tensor_tensor(
                        out=t1[:, 0:w],
                        in0=pG[:, 0:w],
                        in1=dinv_bc[:, bc], op=Alu.mult)
                    t2 = work.tile([128, 512], f32, tag="t2")
                    nc.vector.tensor_tensor(out=t2[:, 0:w],
                                            in0=t1[:, 0:w],
                                            in1=pR[:, 0:w], op=Alu.add)
                    if r in (0, 2):
                        nc.scalar.activation(outT[:, bc], t2[:, 0:w],
                                             Act.Relu,
                                             bias=biasT[:, r:r + 1])
                    else:
                      for hh in range(w // 128):
                        nt = (b * BW) // 128 + hh
                        bc = slice(nt * 128, (nt + 1) * 128)
                        ot = work.tile([128, 128], f32, tag="ot")
                        nc.scalar.activation(
                            ot[:], t2[:, hh * 128:(hh + 1) * 128],
                            Act.Relu, bias=biasT[:, r:r + 1])
                        ph = psR.tile([64, 128], f32, space="PSUM",
                                      tag="R")
                        nc.tensor.matmul(out=ph[:], lhsT=khalf[:],
                                         rhs=ot[:], start=True, stop=True)
                        if r == 1:
                            nc.scalar.activation(h1T[:, bc], ph[:],
                                                 Act.Lrelu, alpha=0.2)
                        else:
                            hm = work.tile([64, 128], f32, tag="hm")
                            nc.scalar.copy(hm[:], ph[:])
                            ht = psP.tile([128, 64], f32, space="PSUM",
                                          tag="zt")
                            nc.tensor.transpose(out=ht[:], in_=hm[:],
                                                identity=identf[:])
                            h2 = work.tile([128, 65], f32, tag="h2")
                            nc.scalar.activation(h2[:, 0:64], ht[:],
                                                 Act.Lrelu, alpha=0.2)
                            nc.vector.memset(h2[:, 64:65], 1.0)
                            selg = work.tile([128, G], f32, tag="selg")
                            nc.vector.tensor_scalar(
                                out=selg[:], in0=iota_g[:],
                                scalar1=bat[:, nt:nt + 1], scalar2=None,
                                op0=Alu.is_equal)
                            nc.tensor.matmul(out=pool_ps[:], lhsT=h2[:],
                                             rhs=selg[:], start=(nt == 0),
                                             stop=(nt == NB - 1))

            # ---------- epilogue ----------
            pools = work.tile([65, G], f32, tag="pools")
            nc.vector.tensor_copy(pools[:], pool_ps[:])
            nc.sync.dma_start(out=ar_in[:], in_=pools[:])
            if STUB_COLLECTIVES:
                nc.sync.dma_start(out=ar_out[:], in_=ar_in[:])
            else:
                nc.gpsimd.collective_compute(
                    "AllReduce", mybir.AluOpType.add,
                    replica_groups=[list(range(NCORES))],
                    ins=[ar_in[:]], outs=[ar_out[:]])
            pall = work.tile([65, G], f32, tag="pall")
            nc.sync.dma_start(out=pall[:], in_=ar_out[:])

            sp = psP.tile([1, G], f32, space="PSUM", tag="poolacc")
            nc.tensor.matmul(out=sp[:], lhsT=fcw[:], rhs=pall[0:64, :],
                             start=True, stop=True)
            cc = work.tile([1, G], f32, tag="cc")
            nc.vector.tensor_scalar(out=cc[:], in0=pall[64:65, :], scalar1=1.0,
                                    scalar2=None, op0=Alu.max)
            rc = work.tile([1, G], f32, tag="rc")
            nc.vector.reciprocal(rc[:], cc[:])
            lg = work.tile([1, G], f32, tag="lg")
            nc.vector.tensor_tensor(out=lg[:], in0=sp[:], in1=rc[:],
                                    op=Alu.mult)
            og = work.tile([1, G], f32, tag="og")
            nc.scalar.activation(og[:], lg[:], Act.Sigmoid,
                                 bias=fcb[0:1, 0:1])
            nc.sync.dma_start(out=out_d[:], in_=og[:])

    nc.compile()
    return nc


def _run(inputs, trace=False, trace_kwargs=None):
    from concourse.bass_utils import run_bass_kernel_spmd

    x = np.asarray(inputs["x"], np.float32)
    edge_index = np.asarray(inputs["edge_index"], np.int32)
    batch = np.asarray(inputs["batch"], np.int32)

    sched, data = _preprocess(x, edge_index, batch)
    wbig, wsml, wbd, biasT, khalf, fcw, fcb = _pack_weights(
        np.asarray(inputs["init_w1"], np.float32),
        np.asarray(inputs["w1"], np.float32),
        np.asarray(inputs["root_w1"], np.float32),
        np.asarray(inputs["b1"], np.float32),
        np.asarray(inputs["init_w2"], np.float32),
        np.asarray(inputs["w2"], np.float32),
        np.asarray(inputs["root_w2"], np.float32),
        np.asarray(inputs["b2"], np.float32),
        np.asarray(inputs["fc_w"], np.float32),
        np.asarray(inputs["fc_b"], np.float32))

    nc = _build(sched)
    in_maps = []
    for c in range(NCORES):
        in_maps.append(dict(
            xT=data["xT"][c], dinvbc=data["dinvbc"][c],
            dinvcol=data["dinvcol"][c], batch=data["batch"][c],
            idx=data["idx"][c], dstr=data["dst"][c],
            iota3=data["iota3"].reshape(128, -1),
            wbig=wbig, wsml=wsml, wbd=wbd, biasT=biasT, khalf=khalf,
            fcw=fcw, fcb=fcb,
        ))
    res = run_bass_kernel_spmd(nc, in_maps, list(range(NCORES)),
                               trace=trace, **(trace_kwargs or {}))
    return np.asarray(res.results[0]["out"]).reshape(G), res


def kernel(x, edge_index, batch, init_w1, w1, root_w1, b1,
           init_w2, w2, root_w2, b2, fc_w, fc_b):
    out, _ = _run(dict(
        x=x, edge_index=edge_index, batch=batch,
        init_w1=init_w1, w1=w1, root_w1=root_w1, b1=b1,
        init_w2=init_w2, w2=w2, root_w2=root_w2, b2=b2,
        fc_w=fc_w, fc_b=fc_b))
    return out


# revision 13
# speedup vs baseline: 1.0288x; 1.0288x over previous
"""ARMA GNN (2x ARMAConv K=2 T=2 + mean-pool + FC head) on 8 TRN2 NeuronCores.

Strategy (graph/data parallel, transposed aggregation), v2.2:
- Factorize the GCN norm: norm[e] = dinv[src]*dinv[dst]; edge aggregation is a
  binary scatter-add of src-prescaled rows, re-scaled on the dst side.
- Host: permute nodes into 784 degree-balanced blocks of 128 (98/core); each
  edge is assigned to its dst 64-block. Per (64-block, src-quartile) lane
  quota = max over cores keeps the SPMD instruction stream identical on all 8
  cores with ~3% padding. Within each gather group (<=8 blocks, split at
  quartile boundaries so quota balancing survives) the node order is w-major
  (w*nblk+bi), so selector one-hots build with fully packed APs (2x DVE mode,
  one is_equal per group per round) and the scatter matmuls write strided
  3D PSUM views whose column order equals node order.
- Per round (4 = 2 convs x T=2): node-major Z matmul per 128-tile (stationary
  = feature-major state slice, moving = weights) -> PSUM->SBUF copy on Act
  with fused per-node dinv scale -> quartile-run-batched ag_in writes -> bf16
  AllGather of Z' [100352,128] -> per (group x quartile) one dma_gather
  (int16 over a <=25600-row table view) -> one matmul per 128-edge chunk:
  stationary = gathered rows, moving = w-major selector slice -> post in
  transposed space: relu(dinv*P^T + root^T + bias^T); Lrelu activation folds
  the leaky-relu between convs.
- Epilogue: segment-sum pooling via one-hot matmuls, AllReduce [65,64],
  FC head + sigmoid on device. Output [64] f32.
- Cost-model status (TimelineSim, collectives stubbed): ~1.17ms sim; DMA
  engines ~94% busy, dominated by the per-edge dma_gather descriptor floor
  (256B granule, ~1.42ns/edge/round); PE sequencer dispatch (~2 instrs per
  128-edge chunk) is co-critical.
"""

import numpy as np
import ml_dtypes

NCORES = 8
N = 100000
F = 128
H = 64
G = 64
S = 12544                 # nodes per core = 98*128
PN = NCORES * S           # 100352 padded global nodes
NB = 98                   # 128-node tiles per core (Z phase / pooling)
QR = PN // 4              # 25088 rows per quartile table view
BW = 64                   # aggregation block width (rows per one-hot)
GSIZE = 8                 # aggregation blocks per gather group (512 nodes)
PADROW = 999.0            # dstrow sentinel for masked lanes
STUB_COLLECTIVES = False  # replace collectives with local DMA (cost-model runs)


def _groups():
    # groups never cross quartile boundaries: the w-major permutation must
    # keep every node in its quartile so pass-2 quota balancing stays valid.
    nbw = S // BW
    qsz0 = ((S + 4 * 128 - 1) // (4 * 128)) * 128
    qb = sorted({min(qsz0 * k // BW, nbw) for k in range(1, 4)} | {nbw})
    gs, b = [], 0
    for e in qb:
        while b < e:
            gs.append(list(range(b, min(b + GSIZE, e))))
            b = min(b + GSIZE, e)
    return gs


def _wmajor_pi():
    """Local-position permutation: within each group of GSIZE 64-blocks the
    node order becomes w-major (w*nblk + bi), so PSUM columns written by the
    w-major selector matmuls coincide with node order."""
    pi = np.empty(S, np.int64)
    groups = _groups()
    for g, blks in enumerate(groups):
        nblk = len(blks)
        base = blks[0] * BW
        idx = np.arange(nblk * BW)
        bi = idx // BW
        w = idx % BW
        pi[base + idx] = base + w * nblk + bi
    return pi


def _preprocess(x, edge_index, batch):
    src, dst = edge_index[0].astype(np.int64), edge_index[1].astype(np.int64)
    deg = np.bincount(dst, minlength=N).astype(np.int64)

    # node permutation: serpentine deal by degree over NCORES*NB bins
    order = np.argsort(-deg, kind="stable")
    order = np.concatenate([order, np.arange(N, PN)])
    nbins = NCORES * NB
    new_of_old = np.empty(PN, np.int64)
    idx = np.arange(PN)
    r = idx // nbins
    pos = idx % nbins
    binid = np.where(r % 2 == 0, pos, nbins - 1 - pos)
    new_of_old[order] = (binid // NB) * S + (binid % NB) * 128 + r

    old_of_new = np.empty(PN, np.int64)
    old_of_new[new_of_old] = np.arange(PN)

    # ---- pass 2: re-deal nodes within their (core, quarter) range to
    # balance per-(64-block, src-quarter) edge-count 4-vectors; quarter
    # membership (and thus every edge's table choice) is fixed by pass 1,
    # so this is non-circular. Shrinks the max-over-cores quotas. ----
    qsz0_p = ((S + 4 * 128 - 1) // (4 * 128)) * 128
    qszs_p = []
    rem_p = S
    for _ in range(4):
        t = min(qsz0_p, rem_p)
        qszs_p.append(t)
        rem_p -= t
    qoffs_p = [0, qszs_p[0], qszs_p[0] + qszs_p[1],
               qszs_p[0] + qszs_p[1] + qszs_p[2]]
    nq_p = len([t for t in qszs_p if t > 0])
    q_e = np.minimum((new_of_old[src] % S) // qsz0_p, nq_p - 1)
    degq = np.zeros((PN, 4), np.int64)
    np.add.at(degq, (dst, q_e), 1)

    new2 = np.empty(PN, np.int64)
    for c in range(NCORES):
        for j in range(nq_p):
            if qszs_p[j] == 0:
                continue
            base = c * S + qoffs_p[j]
            olds = old_of_new[base:base + qszs_p[j]]
            v = degq[olds].astype(np.float64)           # [m, 4]
            nblk = qszs_p[j] // BW
            sums = np.zeros((nblk, 4))
            slots = np.full(nblk, BW, np.int64)
            fill = np.zeros(nblk, np.int64)
            order2 = np.argsort(-v.sum(axis=1), kind="stable")
            for oi in order2:
                cand = sums + v[oi]                      # [nblk, 4]
                score = cand.max(axis=1)
                score[slots == 0] = np.inf
                b = int(np.argmin(score))
                new2[olds[oi]] = base + b * BW + fill[b]
                sums[b] += v[oi]
                slots[b] -= 1
                fill[b] += 1

    # compose the w-major within-group permutation
    pi = _wmajor_pi()
    new_of_old = (new2 // S) * S + pi[new2 % S]
    old_of_new = np.empty(PN, np.int64)
    old_of_new[new_of_old] = np.arange(PN)

    x_pad = np.zeros((PN, F), np.float32)
    x_pad[:N] = x
    x_new = x_pad[old_of_new]
    deg_pad = np.zeros(PN, np.float32)
    deg_pad[:N] = deg.astype(np.float32)
    deg_new = deg_pad[old_of_new]
    batch_pad = np.full(PN, PADROW, np.float32)
    batch_pad[:N] = batch.astype(np.float32)
    batch_new = batch_pad[old_of_new]

    per_core_xT = [np.ascontiguousarray(x_new[c * S:(c + 1) * S].T)
                   .astype(ml_dtypes.bfloat16) for c in range(NCORES)]
    dinv_new = np.where(deg_new > 0,
                        1.0 / np.sqrt(np.maximum(deg_new, 1.0)),
                        0.0).astype(np.float32)
    # dinv broadcast [128, S] bf16 (same value down each column)
    per_core_dinvbc = [
        np.ascontiguousarray(
            np.broadcast_to(dinv_new[c * S:(c + 1) * S][None, :], (128, S)))
        .astype(ml_dtypes.bfloat16) for c in range(NCORES)]
    # dinv node-major per tile: [128, NB] f32, dinv_col[p, t] = dinv[t*128+p]
    per_core_dinvcol = [np.ascontiguousarray(
        dinv_new[c * S:(c + 1) * S].reshape(NB, 128).T) for c in range(NCORES)]
    per_core_batch = [np.ascontiguousarray(batch_new[c * S:(c + 1) * S]
                                           .reshape(NB, 128).T)
                      for c in range(NCORES)]                      # [128, NB]

    # edges
    nsrc = new_of_old[src]
    ndst = new_of_old[dst]
    owner = ndst // S
    groups = _groups()
    ngrp = len(groups)
    nblk_of_g = np.array([len(b) for b in groups], np.int64)
    gbase = np.array([b[0] * BW for b in groups], np.int64)
    loc = ndst % S
    gid = np.searchsorted(gbase, loc, side="right") - 1
    off = loc - gbase[gid]
    nblk_e = nblk_of_g[gid]
    bi = off % nblk_e
    row = (off // nblk_e).astype(np.float32)            # w within 64-block
    blk = gbase[gid] // BW + bi                         # global 64-block id

    qsz0 = ((S + 4 * 128 - 1) // (4 * 128)) * 128
    qszs = []
    rem = S
    for _ in range(4):
        t = min(qsz0, rem)
        qszs.append(t)
        rem -= t
    qoffs = [0, qszs[0], qszs[0] + qszs[1], qszs[0] + qszs[1] + qszs[2]]
    nq = len([t for t in qszs if t > 0])
    local = nsrc % S
    rank = nsrc // S
    q = np.minimum(local // qsz0, nq - 1)
    qsz_a = np.array(qszs, np.int64)
    qoff_a = np.array(qoffs, np.int64)
    # table rows ordered (partition, tile) so each SBUF partition's Z' rows
    # are contiguous in DRAM -> 1KB ag_in write elements (no <512B penalty)
    row_local = ((local % 128) * (qsz_a[q] // 128)
                 + (local // 128 - qoff_a[q] // 128))
    qidx = (rank * qsz_a[q] + row_local).astype(np.int16)

    nbw = S // BW
    key = (owner * nbw + blk) * 4 + q
    counts = np.bincount(key, minlength=NCORES * nbw * 4).reshape(NCORES, nbw, 4)
    quota = counts.max(axis=0)                                     # [nbw, 4]

    layout = []     # per (g,q): blocks, per-block lane offsets, n (=sum quota)
    for g, blks in enumerate(groups):
        for qq in range(4):
            offs, sacc = [], 0
            for b in blks:
                offs.append(sacc)
                sacc += int(quota[b, qq])
            nidx = ((sacc + 127) // 128) * 128
            layout.append(dict(g=g, q=qq, blocks=blks, offs=offs,
                               n=sacc, nidx=nidx))

    # covers ordered by (chunk, block); per chunk the covers are contiguous
    # blocks -> one matmul per chunk spanning its covers.
    covers = []      # per (g,q): list of (c, b, lo, hi, col)
    chunk_mms = []   # per (g,q): list of (c, col0, ncov, bi0)
    ncols_total = 0
    for lay in layout:
        lst = []
        for bi_, b in enumerate(lay["blocks"]):
            lo = lay["offs"][bi_]
            hi = lo + int(quota[b, lay["q"]])
            if hi == lo:
                continue
            c0, c1 = lo // 128, (hi - 1) // 128
            for c in range(c0, c1 + 1):
                lst.append((c, b, lo, hi))
        lst.sort(key=lambda t: (t[0], t[1]))
        lst2 = []
        for i, (c, b, lo, hi) in enumerate(lst):
            lst2.append((c, b, lo, hi, ncols_total + i))
        ncols_total += len(lst2)
        covers.append(lst2)
        mms = []
        i = 0
        while i < len(lst2):
            j = i
            while (j + 1 < len(lst2) and lst2[j + 1][0] == lst2[i][0]):
                j += 1
            c = lst2[i][0]
            bi0 = lay["blocks"].index(lst2[i][1])
            # covered blocks must be contiguous in bi for one matmul
            mms.append((c, lst2[i][4], j - i + 1, bi0))
            i = j + 1
        chunk_mms.append(mms)

    # per-group contiguous cover/col spans for the merged sel build
    grp_cols = []   # per g: (c0, ncv)
    for g in range(ngrp):
        lis = [g * 4 + qq for qq in range(4)]
        cs = [c[0][4] for li in lis if (c := covers[li])]
        ncv = sum(len(covers[li]) for li in lis)
        grp_cols.append((min(cs) if cs else 0, ncv))

    # per-core edge fill
    eorder = np.lexsort((q, blk, owner))
    so_q = q[eorder]
    so_qidx = qidx[eorder]
    so_row = row[eorder]
    keysort = (owner[eorder] * nbw + blk[eorder]) * 4 + so_q
    kstart = np.searchsorted(keysort, np.arange(NCORES * nbw * 4))

    tot_nidx = sum(l["nidx"] for l in layout)
    per_core_idx, per_core_dst = [], []
    for c in range(NCORES):
        idx_parts = []
        dstcols = np.full((128, ncols_total), PADROW, np.float32)
        for li, lay in enumerate(layout):
            flat = np.zeros(lay["nidx"], np.int16)
            lane_dst = np.full(lay["nidx"], PADROW, np.float32)
            for bi_, b in enumerate(lay["blocks"]):
                kk = (c * nbw + b) * 4 + lay["q"]
                s0 = kstart[kk]
                cnt = int(counts[c, b, lay["q"]])
                lo = lay["offs"][bi_]
                flat[lo:lo + cnt] = so_qidx[s0:s0 + cnt]
                lane_dst[lo:lo + cnt] = so_row[s0:s0 + cnt]
            idx_parts.append(np.tile(flat.reshape(-1, 16).T, (8, 1)))
            for (cc, b, lo, hi, col) in covers[li]:
                colv = np.full(128, PADROW, np.float32)
                sl = max(lo, cc * 128)
                e = min(hi, (cc + 1) * 128)
                colv[sl - cc * 128:e - cc * 128] = lane_dst[sl:e]
                dstcols[:, col] = colv
        per_core_idx.append(np.concatenate(idx_parts, axis=1))
        per_core_dst.append(dstcols.astype(ml_dtypes.bfloat16))

    maxcv_g = max(ncv for _, ncv in grp_cols)
    # iota3 [128, 64, maxcv_g] bf16: value w, constant along cv
    iota3 = np.ascontiguousarray(
        np.broadcast_to(np.arange(BW, dtype=np.float32)[None, :, None],
                        (128, BW, maxcv_g))).astype(ml_dtypes.bfloat16)

    sched = dict(layout=layout, covers=covers, chunk_mms=chunk_mms,
                 ncols=ncols_total, tot_nidx=tot_nidx, groups=groups,
                 qszs=qszs, qoffs=qoffs, grp_cols=grp_cols, maxcv_g=maxcv_g)
    data = dict(xT=per_core_xT, dinvbc=per_core_dinvbc,
                dinvcol=per_core_dinvcol, batch=per_core_batch,
                idx=per_core_idx, dst=per_core_dst, iota3=iota3)
    return sched, data


def _pack_weights(iw1, w1, rw1, b1, iw2, w2, rw2, b2, fcw, fcb):
    # wbig [128, 3*128] f32: Wz1 | Wr1 | Wr2   (lhsT, contract = F)
    wz1 = np.concatenate([iw1[0], iw1[1]], axis=1)
    wr1 = np.concatenate([rw1[0, 0], rw1[0, 1]], axis=1)
    wr2 = np.concatenate([rw1[1, 0], rw1[1, 1]], axis=1)
    wbig = np.concatenate([wz1, wr1, wr2], axis=1).astype(np.float32)

    def bd(w):
        m = np.zeros((128, 128), np.float32)
        m[0:64, 0:64] = w[0, 0]
        m[64:128, 64:128] = w[0, 1]
        return m

    wbd = np.concatenate([bd(w1), bd(w2)], axis=1).astype(np.float32)  # [128,256]
    wz3 = np.concatenate([iw2[0], iw2[1]], axis=1)
    wr3 = np.concatenate([rw2[0, 0], rw2[0, 1]], axis=1)
    wr4 = np.concatenate([rw2[1, 0], rw2[1, 1]], axis=1)
    wsml = np.concatenate([wz3, wr3, wr4], axis=1).astype(np.float32)  # [64,384]
    biasT = np.stack([b1[0].ravel(), b1[1].ravel(),
                      b2[0].ravel(), b2[1].ravel()], axis=1)           # [128,4]
    khalf = np.zeros((128, 64), np.float32)
    khalf[0:64] = 0.5 * np.eye(64)
    khalf[64:128] = 0.5 * np.eye(64)
    return (wbig, wsml, wbd, biasT.astype(np.float32), khalf,
            fcw.astype(np.float32), fcb.reshape(1, 1).astype(np.float32))


def _build(sched):
    import concourse.bass as bass
    import concourse.bacc as bacc
    import concourse.mybir as mybir
    import concourse.tile as tile
    from concourse.masks import make_identity

    f32 = mybir.dt.float32
    bf16 = mybir.dt.bfloat16
    i16 = mybir.dt.int16
    Alu = mybir.AluOpType
    Act = mybir.ActivationFunctionType

    layout = sched["layout"]
    covers = sched["covers"]
    chunk_mms = sched["chunk_mms"]
    ncols = sched["ncols"]
    groups = sched["groups"]
    grp_cols = sched["grp_cols"]
    maxcv_g = sched["maxcv_g"]
    maxch = max(l["nidx"] // 128 for l in layout)

    nc = bacc.Bacc("TRN2", target_bir_lowering=False, debug=False,
                   num_devices=1 if STUB_COLLECTIVES else NCORES)

    xT_d = nc.dram_tensor("xT", [128, S], bf16, kind="ExternalInput")
    dinvbc_d = nc.dram_tensor("dinvbc", [128, S], bf16, kind="ExternalInput")
    dinvcol_d = nc.dram_tensor("dinvcol", [128, NB], f32, kind="ExternalInput")
    bat_d = nc.dram_tensor("batch", [128, NB], f32, kind="ExternalInput")
    idx_d = nc.dram_tensor("idx", [128, sched["tot_nidx"] // 16], i16,
                           kind="ExternalInput")
    dstr_d = nc.dram_tensor("dstr", [128, ncols], bf16, kind="ExternalInput")
    iota3_d = nc.dram_tensor("iota3", [128, BW * maxcv_g], bf16,
                             kind="ExternalInput")
    wbig_d = nc.dram_tensor("wbig", [128, 384], f32, kind="ExternalInput")
    wsml_d = nc.dram_tensor("wsml", [64, 384], f32, kind="ExternalInput")
    wbd_d = nc.dram_tensor("wbd", [128, 256], f32, kind="ExternalInput")
    biasT_d = nc.dram_tensor("biasT", [128, 4], f32, kind="ExternalInput")
    khalf_d = nc.dram_tensor("khalf", [128, 64], f32, kind="ExternalInput")
    fcw_d = nc.dram_tensor("fcw", [64, 1], f32, kind="ExternalInput")
    fcb_d = nc.dram_tensor("fcb", [1, 1], f32, kind="ExternalInput")
    out_d = nc.dram_tensor("out", [1, G], f32, kind="ExternalOutput")

    qszs = sched["qszs"]
    qoffs = sched["qoffs"]
    ag_in = [[nc.dram_tensor(f"ag_in{r}_{j}", [qszs[j], 128], bf16)
              if qszs[j] > 0 else None for j in range(4)] for r in range(4)]
    ag_out = [[nc.dram_tensor(f"ag_out{r}_{j}", [NCORES * qszs[j], 128], bf16,
                              addr_space="Shared")
               if qszs[j] > 0 else None for j in range(4)] for r in range(4)]
    ar_in = nc.dram_tensor("ar_in", [65, G], f32)
    ar_out = nc.dram_tensor("ar_out", [65, G], f32, addr_space="Shared")

    with tile.TileContext(nc) as tc:
        with (
            tc.tile_pool(name="big", bufs=1) as big,
            tc.tile_pool(name="land", bufs=2) as land,
            tc.tile_pool(name="selpool", bufs=2) as selp,
            tc.tile_pool(name="work", bufs=4) as work,
            tc.tile_pool(name="zsb", bufs=3) as zsbp,
            tc.tile_pool(name="psG", bufs=2, space="PSUM") as psG,
            tc.tile_pool(name="psR", bufs=2, space="PSUM") as psR,
            tc.tile_pool(name="psZ", bufs=2, space="PSUM") as psZ,
            tc.tile_pool(name="psP", bufs=1, space="PSUM") as psP,
        ):
            # ---------- prologue ----------
            xT = big.tile([128, S], bf16, tag="xT")
            for pc in range(4):
                sl = slice(pc * (S // 4), (pc + 1) * (S // 4))
                nc.sync.dma_start(out=xT[:, sl], in_=xT_d[:, sl])
            outT = big.tile([128, S], bf16, tag="outT")
            h1T = big.tile([64, S], bf16, tag="h1T")
            dstr = big.tile([128, ncols], bf16, tag="dstr")
            nc.sync.dma_start(out=dstr[:], in_=dstr_d[:])
            iota3 = big.tile([128, BW, maxcv_g], bf16, tag="iota3")
            nc.sync.dma_start(
                out=bass.AP(iota3[:].tensor, iota3[:].offset,
                            [[iota3[:].ap[0][0], 128], [1, BW * maxcv_g]]),
                in_=iota3_d[:])
            bat = big.tile([128, NB], f32, tag="bat")
            nc.sync.dma_start(out=bat[:], in_=bat_d[:])
            dinv_col = big.tile([128, NB], f32, tag="dinvcol")
            nc.sync.dma_start(out=dinv_col[:], in_=dinvcol_d[:])
            wbig = big.tile([128, 384], f32, tag="wbig")
            nc.sync.dma_start(out=wbig[:], in_=wbig_d[:])
            wsml = big.tile([64, 384], f32, tag="wsml")
            nc.sync.dma_start(out=wsml[:], in_=wsml_d[:])
            wbd = big.tile([128, 256], f32, tag="wbd")
            nc.sync.dma_start(out=wbd[:], in_=wbd_d[:])
            biasT = big.tile([128, 4], f32, tag="biasT")
            nc.sync.dma_start(out=biasT[:], in_=biasT_d[:])
            khalf = big.tile([128, 64], f32, tag="khalf")
            nc.sync.dma_start(out=khalf[:], in_=khalf_d[:])
            fcw = big.tile([64, 1], f32, tag="fcw")
            nc.sync.dma_start(out=fcw[:], in_=fcw_d[:])
            fcb = big.tile([1, 1], f32, tag="fcb")
            nc.sync.dma_start(out=fcb[:], in_=fcb_d[:])
            wbig_b = big.tile([128, 384], bf16, tag="wbig_b")
            nc.vector.tensor_copy(wbig_b[:], wbig[:])
            wsml_b = big.tile([64, 384], bf16, tag="wsml_b")
            nc.vector.tensor_copy(wsml_b[:], wsml[:])
            wbd_b = big.tile([128, 256], bf16, tag="wbd_b")
            nc.vector.tensor_copy(wbd_b[:], wbd[:])

            iota_i = big.tile([128, 128], mybir.dt.int32, tag="iota_i")
            nc.gpsimd.iota(iota_i[:], pattern=[[1, 128]], base=0,
                           channel_multiplier=0)
            iota_g = big.tile([128, G], f32, tag="iota_g")
            nc.vector.tensor_copy(iota_g[:], iota_i[:, :G])
            identf = big.tile([64, 64], f32, tag="identf")
            make_identity(nc, identf[:])

            dinv_bc = big.tile([128, S], bf16, tag="dinv_bc")
            nc.sync.dma_start(out=dinv_bc[:], in_=dinvbc_d[:])

            pool_ps = psP.tile([65, G], f32, space="PSUM", tag="poolacc")

            # ---------- rounds ----------
            for r in range(4):
                # Z phase: per 128-node tile, node-major Z matmul; PSUM->SBUF
                # on Act with fused dinv scale; quad-batched ag_in writes.
                for b0 in range(0, NB, 4):
                    nt4 = min(4, NB - b0)
                    zq = zsbp.tile([128, 4, 128], bf16, tag="zq")
                    for j in range(nt4):
                        t = b0 + j
                        tc_sl = slice(t * 128, (t + 1) * 128)
                        zp = psZ.tile([128, 128], f32, space="PSUM", tag="z")
                        if r == 0:
                            nc.tensor.matmul(out=zp[:], lhsT=xT[:, tc_sl],
                                             rhs=wbig_b[:, 0:128],
                                             start=True, stop=True)
                        elif r == 2:
                            nc.tensor.matmul(out=zp[:], lhsT=h1T[:, tc_sl],
                                             rhs=wsml_b[:, 0:128],
                                             start=True, stop=True)
                        else:
                            wof = 0 if r == 1 else 128
                            nc.tensor.matmul(out=zp[:], lhsT=outT[:, tc_sl],
                                             rhs=wbd_b[:, wof:wof + 128],
                                             start=True, stop=True)
                        nc.scalar.activation(zq[:, j, :], zp[:], Act.Copy,
                                             scale=dinv_col[:, t:t + 1])
                    # one DMA per quartile-run: table rows ordered (p, tile)
                    # so each partition writes a contiguous 128*nrun run.
                    j0 = 0
                    while j0 < nt4:
                        jq = min(((b0 + j0) * 128) // max(qszs[0], 1), 3)
                        j1 = j0
                        while (j1 + 1 < nt4 and
                               min(((b0 + j1 + 1) * 128) // max(qszs[0], 1), 3)
                               == jq):
                            j1 += 1
                        ntq = qszs[jq] // 128
                        t0 = (b0 + j0) - qoffs[jq] // 128
                        nrun = j1 - j0 + 1
                        tgt = ag_in[r][jq]
                        out_ap = bass.AP(
                            tgt[:].tensor, tgt[:].offset + t0 * 128,
                            [[ntq * 128, 128], [128, nrun], [1, 128]])
                        nc.sync.dma_start(out=out_ap, in_=zq[:, j0:j1 + 1, :])
                        j0 = j1 + 1

                    # fire the AllGather for any quartile completed by this
                    # quad, so it overlaps the rest of the Z phase.
                    for j in range(4):
                        if ag_in[r][j] is None:
                            continue
                        lastt = (qoffs[j] + qszs[j]) // 128 - 1
                        if not (b0 <= lastt < b0 + nt4):
                            continue
                        if STUB_COLLECTIVES:
                            # pair-row views (512B elems) avoid the <512B
                            # DMA penalty; same bytes either way.
                            src = ag_in[r][j][:]
                            dst = ag_out[r][j][:]
                            nc.sync.dma_start(
                                out=bass.AP(dst.tensor, dst.offset,
                                            [[256, qszs[j] // 2], [1, 256]]),
                                in_=bass.AP(src.tensor, src.offset,
                                            [[256, qszs[j] // 2], [1, 256]]))
                        else:
                            nc.gpsimd.collective_compute(
                                "AllGather", mybir.AluOpType.bypass,
                                replica_groups=[list(range(NCORES))],
                                ins=[ag_in[r][j][:]], outs=[ag_out[r][j][:]])

                ioff16 = 0
                for g, blks in enumerate(groups):
                    nb_g = len(blks)
                    pG = psG.tile([128, GSIZE * BW], f32, space="PSUM",
                                  tag="Gp")
                    nc.vector.memset(pG[:, 0:nb_g * BW], 0.0)
                    gn16 = sum(layout[g * 4 + qq]["nidx"] // 16
                               for qq in range(4))
                    it_g = work.tile([128, 4 * (maxch * 128) // 16], i16,
                                     tag="idxg")
                    if gn16 > 0:
                        nc.sync.dma_start(out=it_g[:, 0:gn16],
                                          in_=idx_d[:, ioff16:ioff16 + gn16])

                    # merged w-major selector build for the whole group:
                    # sel[p, w, cv] = (w == dstr[p, c0+cv]); all APs packed.
                    c0g, ncvg = grp_cols[g]
                    sel = selp.tile([128, BW, maxcv_g], bf16, tag="sel")
                    if ncvg:
                        dbc = dstr[:, c0g:c0g + ncvg]
                        dbc3 = bass.AP(dbc.tensor, dbc.offset,
                                       [[dbc.ap[0][0], 128], [0, BW],
                                        [1, ncvg]])
                        nc.vector.tensor_tensor(out=sel[:, :, 0:ncvg],
                                                in0=iota3[:, :, 0:ncvg],
                                                in1=dbc3, op=Alu.is_equal)

                    goff16 = 0
                    gts = []
                    for qq in range(4):
                        li = g * 4 + qq
                        lay = layout[li]
                        nch = lay["nidx"] // 128
                        n16 = lay["nidx"] // 16
                        gt = land.tile([128, maxch, 128], bf16, tag=f"g{qq}")
                        if nch > 0:
                            nc.gpsimd.dma_gather(
                                out_ap=gt[:, 0:nch, :],
                                in_ap=ag_out[r][lay["q"]][:],
                                idxs_ap=it_g[:, goff16:goff16 + n16],
                                num_idxs=lay["nidx"],
                                num_idxs_reg=lay["nidx"],
                                elem_size=128,
                                single_packet=False,
                            )
                        goff16 += n16
                        gts.append(gt)
                        ioff16 += n16

                    pGv = pG[:]
                    for qq in range(4):
                        li = g * 4 + qq
                        if not covers[li]:
                            continue
                        for (c, col, ncov, bi0) in chunk_mms[li]:
                            out_ap = bass.AP(
                                pGv.tensor, pGv.offset + bi0,
                                [[pGv.ap[0][0], 128], [nb_g, BW], [1, ncov]])
                            nc.tensor.matmul(
                                out=out_ap,
                                lhsT=gts[qq][:, c, :],
                                rhs=sel[:, :, col - c0g:col - c0g + ncov],
                                start=False, stop=False,
                                skip_group_check=True)

                    b = blks[0]
                    w = nb_g * BW
                    bc = slice(b * BW, b * BW + w)
                    pR = psR.tile([128, 512], f32, space="PSUM", tag="R")
                    if r <= 1:
                        nc.tensor.matmul(
                            out=pR[:, 0:w],
                            lhsT=wbig_b[:, 128 + r * 128:256 + r * 128],
                            rhs=xT[:, bc], start=True, stop=True)
                    else:
                        wof = 128 + (r - 2) * 128
                        nc.tensor.matmul(
                            out=pR[:, 0:w], lhsT=wsml_b[:, wof:wof + 128],
                            rhs=h1T[:, bc], start=True, stop=True)
                    t1 = work.tile([128, 512], f32, tag="t1")
                    nc.vector.tensor_tensor(
                        out=t1[:, 0:w],
                        in0=pG[:, 0:w],
                        in1=dinv_bc[:, bc], op=Alu.mult)
                    t2 = work.tile([128, 512], f32, tag="t2")
                    nc.vector.tensor_tensor(out=t2[:, 0:w],
                                            in0=t1[:, 0:w],
                                            in1=pR[:, 0:w], op=Alu.add)
                    if r in (0, 2):
                        nc.scalar.activation(outT[:, bc], t2[:, 0:w],
                                             Act.Relu,
                                             bias=biasT[:, r:r + 1])
                    else:
                      for hh in range(w // 128):
                        nt = (b * BW) // 128 + hh
                        bc = slice(nt * 128, (nt + 1) * 128)
                        ot = work.tile([128, 128], f32, tag="ot")
                        nc.scalar.activation(
                            ot[:], t2[:, hh * 128:(hh + 1) * 128],
                            Act.Relu, bias=biasT[:, r:r + 1])
                        ph = psR.tile([64, 128], f32, space="PSUM",
                                      tag="R")
                        nc.tensor.matmul(out=ph[:], lhsT=khalf[:],
                                         rhs=ot[:], start=True, stop=True)
                        if r == 1:
                            nc.scalar.activation(h1T[:, bc], ph[:],
                                                 Act.Lrelu, alpha=0.2)
                        else:
                            hm = work.tile([64, 128], f32, tag="hm")
                            nc.scalar.copy(hm[:], ph[:])
                            ht = psP.tile([128, 64], f32, space="PSUM",
                                          tag="zt")
                            nc.tensor.transpose(out=ht[:], in_=hm[:],
                                                identity=identf[:])
                            h2 = work.tile([128, 65], f32, tag="h2")
                            nc.scalar.activation(h2[:, 0:64], ht[:],
                                                 Act.Lrelu, alpha=0.2)
                            nc.vector.memset(h2[:, 64:65], 1.0)
                            selg = work.tile([128, G], f32, tag="selg")
                            nc.vector.tensor_scalar(
                                out=selg[:], in0=iota_g[:],
                                scalar1=bat[:, nt:nt + 1], scalar2=None,
                                op0=Alu.is_equal)
                            nc.tensor.matmul(out=pool_ps[:], lhsT=h2[:],
                                             rhs=selg[:], start=(nt == 0),
                                             stop=(nt == NB - 1))

            # ---------- epilogue ----------
            pools = work.tile([65, G], f32, tag="pools")
            nc.vector.tensor_copy(pools[:], pool_ps[:])
            nc.sync.dma_start(out=ar_in[:], in_=pools[:])
            if STUB_COLLECTIVES:
                nc.sync.dma_start(out=ar_out[:], in_=ar_in[:])
            else:
                nc.gpsimd.collective_compute(
                    "AllReduce", mybir.AluOpType.add,
                    replica_groups=[list(range(NCORES))],
                    ins=[ar_in[:]], outs=[ar_out[:]])
            pall = work.tile([65, G], f32, tag="pall")
            nc.sync.dma_start(out=pall[:], in_=ar_out[:])

            sp = psP.tile([1, G], f32, space="PSUM", tag="poolacc")
            nc.tensor.matmul(out=sp[:], lhsT=fcw[:], rhs=pall[0:64, :],
                             start=True, stop=True)
            cc = work.tile([1, G], f32, tag="cc")
            nc.vector.tensor_scalar(out=cc[:], in0=pall[64:65, :], scalar1=1.0,
                                    scalar2=None, op0=Alu.max)
            rc = work.tile([1, G], f32, tag="rc")
            nc.vector.reciprocal(rc[:], cc[:])
            lg = work.tile([1, G], f32, tag="lg")
            nc.vector.tensor_tensor(out=lg[:], in0=sp[:], in1=rc[:],
                                    op=Alu.mult)
            og = work.tile([1, G], f32, tag="og")
            nc.scalar.activation(og[:], lg[:], Act.Sigmoid,
                                 bias=fcb[0:1, 0:1])
            nc.sync.dma_start(out=out_d[:], in_=og[:])

    nc.compile()
    return nc


def _run(inputs, trace=False, trace_kwargs=None):
    from concourse.bass_utils import run_bass_kernel_spmd

    x = np.asarray(inputs["x"], np.float32)
    edge_index = np.asarray(inputs["edge_index"], np.int32)
    batch = np.asarray(inputs["batch"], np.int32)

    sched, data = _preprocess(x, edge_index, batch)
    wbig, wsml, wbd, biasT, khalf, fcw, fcb = _pack_weights(
        np.asarray(inputs["init_w1"], np.float32),
        np.asarray(inputs["w1"], np.float32),
        np.asarray(inputs["root_w1"], np.float32),
        np.asarray(inputs["b1"], np.float32),
        np.asarray(inputs["init_w2"], np.float32),
        np.asarray(inputs["w2"], np.float32),
        np.asarray(inputs["root_w2"], np.float32),
        np.asarray(inputs["b2"], np.float32),
        np.asarray(inputs["fc_w"], np.float32),
        np.asarray(inputs["fc_b"], np.float32))

    nc = _build(sched)
    in_maps = []
    for c in range(NCORES):
        in_maps.append(dict(
            xT=data["xT"][c], dinvbc=data["dinvbc"][c],
            dinvcol=data["dinvcol"][c], batch=data["batch"][c],
            idx=data["idx"][c], dstr=data["dst"][c],
            iota3=data["iota3"].reshape(128, -1),
            wbig=wbig, wsml=wsml, wbd=wbd, biasT=biasT, khalf=khalf,
            fcw=fcw, fcb=fcb,
        ))
    res = run_bass_kernel_spmd(nc, in_maps, list(range(NCORES)),
                               trace=trace, **(trace_kwargs or {}))
    return np.asarray(res.results[0]["out"]).reshape(G), res


def kernel(x, edge_index, batch, init_w1, w1, root_w1, b1,
           init_w2, w2, root_w2, b2, fc_w, fc_b):
    out, _ = _run(dict(
        x=x, edge_index=edge_index, batch=batch,
        init_w1=init_w1, w1=w1, root_w1=root_w1, b1=b1,
        init_w2=init_w2, w2=w2, root_w2=root_w2, b2=b2,
        fc_w=fc_w, fc_b=fc_b))
    return out
